# revision 12
# baseline (speedup 1.0000x reference)
"""Trainium2 Bass kernel for nn_MessageFunctionForEvent (GNN message function).

Math: the reference is
    em  = W_e2m @ e_wv[b] + b_e2m          (per-node Linear on edge features)
    nw  = W_n2m @ h_w[b]  + b_n2m          (per-node Linear on node features)
    nv  = W_n2m @ h_v[b]  + b_n2m          (node-level, no n axis)
    msg = Wa @ em + Wb @ nw + (Wc @ nv + b_resize)[:, None]
which collapses (precomposing the tiny 128x128 weights on host) to
    msg[b, :, n] = A @ e_wv[b, :, n] + Bm @ h_w[b, :, n] + c[b]
with A = Wa@W_e2m, Bm = Wb@W_n2m, c[b] = Wa@b_e2m + Wb@b_n2m + Wc@nv[b] + b_resize.

Device kernel: stream e/h column chunks HBM->SBUF in fp16 (host casts; halves
HBM traffic vs fp32 — the kernel is HBM-bound at ~420 GB/s/core), e on the
sync HWDGE queue and h on the gpsimd queue, weights on the scalar queue so
the input streams start immediately. Each chunk is 4 x 500-col PSUM tiles
(banks cycle 0-3/4-7 with an 8-buf pool: two chunks of decoupling, so the PE
never stalls on bank reuse and its DVFS clock stays at the fast p-state,
issuing a 500-col matmul every ~210ns). Two accumulated fp16 matmuls per tile
(A-passes of a chunk first — they only need e — then B-passes as h lands).
Bias-add + fp32 PSUM -> fp16 SBUF cast on VectorE tensor_scalar_add; fp16
outputs DMA per chunk on the dedicated scalar queue (host upcasts to fp32).
Every DMA stream owns its queue — mixing out-triggers into the input queues
head-of-line blocks the prefetch stream behind semaphore waits (measured
+13us). Chunks taper small at the start (outputs flow early, filling HBM)
and end (short drain). fp16 rounding costs ~3e-4 normed rel err vs the
2e-2 gate.
Sharding: batch axis (16 batches -> 2 per core), zero host re-layout.
"""

import sys

import numpy as np

try:
    from concourse import bacc, mybir
except ImportError:  # bare environment: fall back to the in-container repo
    sys.path.append("/opt/trn_rl_repo")
    from concourse import bacc, mybir
import concourse.tile as tile
from concourse.bass_utils import run_bass_kernel_spmd

B, F, N = 16, 128, 20000
NCORES = 8
BPC = B // NCORES          # batches per core
NT = 500                   # columns per matmul (fits one 2KB fp32 PSUM bank)

_cached_nc = None


def _chunks_for(b):
    # per-batch column chunk sizes (each a multiple of NT, <= 4*NT so PSUM
    # banks cycle cleanly). First batch ramps up small so the first outputs
    # (and the out DMA stream) start as early as possible; last batch ramps
    # down so the pipeline drains with small quanta.
    mid = [2000] * 8
    if b == 0:
        sched = [500, 1000, 1500] + mid + [1000]
    elif b == BPC - 1:
        sched = mid + [2000, 1000, 500, 500]
    else:
        sched = mid + [2000, 2000]
    assert sum(sched) == N, (b, sum(sched))
    return sched


def _build():
    global _cached_nc
    if _cached_nc is not None:
        return _cached_nc
    f32 = mybir.dt.float32
    f16 = mybir.dt.float16
    nc = bacc.Bacc("TRN2", target_bir_lowering=False, debug=False,
                   num_devices=NCORES)
    e_d = nc.dram_tensor("e_wv", (BPC, F, N), f16, kind="ExternalInput").ap()
    h_d = nc.dram_tensor("h_w", (BPC, F, N), f16, kind="ExternalInput").ap()
    at_d = nc.dram_tensor("at", (F, F), f16, kind="ExternalInput").ap()
    bt_d = nc.dram_tensor("bt", (F, F), f16, kind="ExternalInput").ap()
    c_d = nc.dram_tensor("c", (F, BPC), f32, kind="ExternalInput").ap()
    o_d = nc.dram_tensor("msg", (BPC, F, N), f16, kind="ExternalOutput").ap()

    with tile.TileContext(nc) as tc:
        with tc.tile_pool(name="w", bufs=1) as wp, \
             tc.tile_pool(name="eh", bufs=4) as ehp, \
             tc.tile_pool(name="out", bufs=4) as opp, \
             tc.tile_pool(name="ps", bufs=8, space="PSUM") as psp:
            at_t = wp.tile([F, F], f16)
            nc.scalar.dma_start(at_t[:], at_d[:])
            bt_t = wp.tile([F, F], f16)
            nc.scalar.dma_start(bt_t[:], bt_d[:])
            c_t = wp.tile([F, BPC], f32)
            nc.scalar.dma_start(c_t[:], c_d[:])
            for b in range(BPC):
                n0 = 0
                for cs in _chunks_for(b):
                    sl = slice(n0, n0 + cs)
                    e_t = ehp.tile([F, cs], f16, tag="e")
                    h_t = ehp.tile([F, cs], f16, tag="h")
                    o_t = opp.tile([F, cs], f16, tag="o")
                    nc.sync.dma_start(e_t[:], e_d[b, :, sl])
                    nc.gpsimd.dma_start(h_t[:], h_d[b, :, sl])
                    nk = cs // NT
                    ps_ts = []
                    for k in range(nk):
                        ksl = slice(k * NT, (k + 1) * NT)
                        ps_t = psp.tile([F, NT], f32, tag="ps")
                        ps_ts.append(ps_t)
                        nc.tensor.matmul(ps_t[:], at_t[:], e_t[:, ksl],
                                         start=True, stop=False)
                    for k in range(nk):
                        ksl = slice(k * NT, (k + 1) * NT)
                        nc.tensor.matmul(ps_ts[k][:], bt_t[:], h_t[:, ksl],
                                         start=False, stop=True)
                        nc.vector.tensor_scalar_add(
                            o_t[:, ksl], ps_ts[k][:], c_t[:, b:b + 1])
                    nc.scalar.dma_start(o_d[b, :, sl], o_t[:])
                    n0 += cs
    nc.finalize()
    _cached_nc = nc
    return nc


def _prepare_in_maps(h_w, h_v, e_wv, W_e2m, b_e2m, W_n2m, b_n2m,
                     W_resize, b_resize):
    f64 = np.float64
    M = F
    Wa = W_resize[:, :M].astype(f64)
    Wb = W_resize[:, M:2 * M].astype(f64)
    Wc = W_resize[:, 2 * M:].astype(f64)
    A = Wa @ W_e2m.astype(f64)
    Bm = Wb @ W_n2m.astype(f64)
    nv = h_v.astype(f64) @ W_n2m.astype(f64).T + b_n2m.astype(f64)
    c = (Wa @ b_e2m.astype(f64) + Wb @ b_n2m.astype(f64)
         + nv @ Wc.T + b_resize.astype(f64))          # [B, M]
    AT = np.ascontiguousarray(A.T).astype(np.float16)
    BT = np.ascontiguousarray(Bm.T).astype(np.float16)
    cT = np.ascontiguousarray(c.T).astype(np.float32)  # [M, B]

    e16 = e_wv.astype(np.float16)
    h16 = h_w.astype(np.float16)
    in_maps = []
    for cid in range(NCORES):
        bs = slice(cid * BPC, (cid + 1) * BPC)
        in_maps.append({
            "e_wv": e16[bs],
            "h_w": h16[bs],
            "at": AT,
            "bt": BT,
            "c": np.ascontiguousarray(cT[:, bs]),
        })
    return in_maps


def kernel(**inputs):
    args = {k: np.asarray(inputs[k], dtype=np.float32)
            for k in ("h_w", "h_v", "e_wv", "W_e2m", "b_e2m", "W_n2m",
                      "b_n2m", "W_resize", "b_resize")}
    in_maps = _prepare_in_maps(**args)
    nc = _build()
    res = run_bass_kernel_spmd(nc, in_maps, core_ids=list(range(NCORES)))
    out = np.concatenate([r["msg"] for r in res.results], axis=0)
    return out.astype(np.float32)


# revision 13
# speedup vs baseline: 1.0009x; 1.0009x over previous
"""Trainium2 Bass kernel for nn_MessageFunctionForEvent (GNN message function).

Math: the reference is
    em  = W_e2m @ e_wv[b] + b_e2m          (per-node Linear on edge features)
    nw  = W_n2m @ h_w[b]  + b_n2m          (per-node Linear on node features)
    nv  = W_n2m @ h_v[b]  + b_n2m          (node-level, no n axis)
    msg = Wa @ em + Wb @ nw + (Wc @ nv + b_resize)[:, None]
which collapses (precomposing the tiny 128x128 weights on host) to
    msg[b, :, n] = A @ e_wv[b, :, n] + Bm @ h_w[b, :, n] + c[b]
with A = Wa@W_e2m, Bm = Wb@W_n2m, c[b] = Wa@b_e2m + Wb@b_n2m + Wc@nv[b] + b_resize.

The kernel is HBM-bound (~430 GB/s/core across 16 DMA engines); fp16 staging
(host casts, ~3e-4 normed rel err vs the 2e-2 gate) halves traffic vs fp32:
30.7 MB/core -> ~71 us of pure transfer.

Inventory design: a core's whole fp16 input (2 batches x 128 x 20000 x 2
tensors = 160 KB/partition) fits in SBUF, so ALL input-piece DMAs are issued
up front into never-recycled tiles — e pieces on the sync queue, h pieces on
the gpsimd queue — streaming flat out with no consumer backpressure. The PE
then runs long uninterrupted matmul bursts (its DVFS clock ramps to the fast
p-state only after ~3 us of continuous work, halving per-matmul time), two
accumulated fp16 matmuls per 500-col PSUM tile. PSUM bank sets cycle 0-3/4-7
per 2000-col chunk (8-buf pool). The fp32 PSUM -> fp16 SBUF bias-add drain
alternates Vector/Scalar per chunk (same-parity bank recycling keeps
semaphore chains simple). Out DMAs trigger per chunk, alternating sync/
gpsimd — safe from head-of-line blocking because every input trigger is
already queued ahead of them. Host upcasts the fp16 output to fp32.
Sharding: batch axis (16 batches -> 2 per core), zero host re-layout.
"""

import sys

import numpy as np

try:
    from concourse import bacc, mybir
except ImportError:  # bare environment: fall back to the in-container repo
    sys.path.append("/opt/trn_rl_repo")
    from concourse import bacc, mybir
import concourse.tile as tile
from concourse.bass_utils import run_bass_kernel_spmd

B, F, N = 16, 128, 20000
NCORES = 8
BPC = B // NCORES          # batches per core
NT = 500                   # columns per matmul (fits one 2KB fp32 PSUM bank)
PC = 2500                  # columns per input DMA piece (5000B/partition)
NP = N // PC               # input pieces per batch (8)
CK = 2000                  # columns per output chunk (4 PSUM tiles)
NC = N // CK               # chunks per batch (10)

_cached_nc = None


def _build():
    global _cached_nc
    if _cached_nc is not None:
        return _cached_nc
    f32 = mybir.dt.float32
    f16 = mybir.dt.float16
    nc = bacc.Bacc("TRN2", target_bir_lowering=False, debug=False,
                   num_devices=NCORES)
    e_d = nc.dram_tensor("e_wv", (BPC, F, N), f16, kind="ExternalInput").ap()
    h_d = nc.dram_tensor("h_w", (BPC, F, N), f16, kind="ExternalInput").ap()
    at_d = nc.dram_tensor("at", (F, F), f16, kind="ExternalInput").ap()
    bt_d = nc.dram_tensor("bt", (F, F), f16, kind="ExternalInput").ap()
    c_d = nc.dram_tensor("c", (F, BPC), f32, kind="ExternalInput").ap()
    o_d = nc.dram_tensor("msg", (BPC, F, N), f16, kind="ExternalOutput").ap()

    ident = mybir.ActivationFunctionType.Identity

    with tile.TileContext(nc) as tc:
        with tc.tile_pool(name="w", bufs=1) as wp, \
             tc.tile_pool(name="inv", bufs=BPC * NP) as inv, \
             tc.tile_pool(name="out", bufs=6) as opp, \
             tc.tile_pool(name="ps", bufs=8, space="PSUM") as psp:
            at_t = wp.tile([F, F], f16)
            nc.scalar.dma_start(at_t[:], at_d[:])
            bt_t = wp.tile([F, F], f16)
            nc.scalar.dma_start(bt_t[:], bt_d[:])
            c_t = wp.tile([F, BPC], f32)
            nc.scalar.dma_start(c_t[:], c_d[:])

            # whole-input prefetch: every piece gets its own SBUF tile slot
            # (bufs == piece count, so slots are never recycled and the DMA
            # streams run with zero backpressure).
            e_ts, h_ts = [], []
            for b in range(BPC):
                for p in range(NP):
                    sl = slice(p * PC, (p + 1) * PC)
                    e_t = inv.tile([F, PC], f16, tag="e")
                    nc.sync.dma_start(e_t[:], e_d[b, :, sl])
                    e_ts.append(e_t)
                    h_t = inv.tile([F, PC], f16, tag="h")
                    nc.gpsimd.dma_start(h_t[:], h_d[b, :, sl])
                    h_ts.append(h_t)

            ci = 0  # global chunk counter
            for b in range(BPC):
                for c in range(NC):
                    o_t = opp.tile([F, CK], f16, tag="o")
                    ps_ts = []
                    for k in range(CK // NT):
                        t = c * (CK // NT) + k          # batch tile idx
                        g = b * NP + (t * NT) // PC     # global piece idx
                        ksl = slice((t * NT) % PC, (t * NT) % PC + NT)
                        ps_t = psp.tile([F, NT], f32, tag="ps")
                        ps_ts.append((ps_t, g, ksl))
                        nc.tensor.matmul(ps_t[:], at_t[:], e_ts[g][:, ksl],
                                         start=True, stop=False)
                    for k, (ps_t, g, ksl) in enumerate(ps_ts):
                        nc.tensor.matmul(ps_t[:], bt_t[:], h_ts[g][:, ksl],
                                         start=False, stop=True)
                        osl = slice(k * NT, (k + 1) * NT)
                        if ci % 2 == 0:
                            nc.vector.tensor_scalar_add(
                                o_t[:, osl], ps_t[:], c_t[:, b:b + 1])
                        else:
                            nc.scalar.activation(
                                o_t[:, osl], ps_t[:], ident,
                                bias=c_t[:, b:b + 1], scale=1.0)
                    oq = nc.sync if ci % 2 == 0 else nc.gpsimd
                    oq.dma_start(o_d[b, :, c * CK:(c + 1) * CK], o_t[:])
                    ci += 1
    nc.finalize()
    _cached_nc = nc
    return nc


def _prepare_in_maps(h_w, h_v, e_wv, W_e2m, b_e2m, W_n2m, b_n2m,
                     W_resize, b_resize):
    f64 = np.float64
    M = F
    Wa = W_resize[:, :M].astype(f64)
    Wb = W_resize[:, M:2 * M].astype(f64)
    Wc = W_resize[:, 2 * M:].astype(f64)
    A = Wa @ W_e2m.astype(f64)
    Bm = Wb @ W_n2m.astype(f64)
    nv = h_v.astype(f64) @ W_n2m.astype(f64).T + b_n2m.astype(f64)
    c = (Wa @ b_e2m.astype(f64) + Wb @ b_n2m.astype(f64)
         + nv @ Wc.T + b_resize.astype(f64))          # [B, M]
    AT = np.ascontiguousarray(A.T).astype(np.float16)
    BT = np.ascontiguousarray(Bm.T).astype(np.float16)
    cT = np.ascontiguousarray(c.T).astype(np.float32)  # [M, B]

    e16 = e_wv.astype(np.float16)
    h16 = h_w.astype(np.float16)
    in_maps = []
    for cid in range(NCORES):
        bs = slice(cid * BPC, (cid + 1) * BPC)
        in_maps.append({
            "e_wv": e16[bs],
            "h_w": h16[bs],
            "at": AT,
            "bt": BT,
            "c": np.ascontiguousarray(cT[:, bs]),
        })
    return in_maps


def kernel(**inputs):
    args = {k: np.asarray(inputs[k], dtype=np.float32)
            for k in ("h_w", "h_v", "e_wv", "W_e2m", "b_e2m", "W_n2m",
                      "b_n2m", "W_resize", "b_resize")}
    in_maps = _prepare_in_maps(**args)
    nc = _build()
    res = run_bass_kernel_spmd(nc, in_maps, core_ids=list(range(NCORES)))
    out = np.concatenate([r["msg"] for r in res.results], axis=0)
    return out.astype(np.float32)


# revision 14
# speedup vs baseline: 1.0060x; 1.0050x over previous
"""Trainium2 Bass kernel for nn_MessageFunctionForEvent (GNN message function).

Math: the reference is
    em  = W_e2m @ e_wv[b] + b_e2m          (per-node Linear on edge features)
    nw  = W_n2m @ h_w[b]  + b_n2m          (per-node Linear on node features)
    nv  = W_n2m @ h_v[b]  + b_n2m          (node-level, no n axis)
    msg = Wa @ em + Wb @ nw + (Wc @ nv + b_resize)[:, None]
which collapses (precomposing the tiny 128x128 weights on host) to
    msg[b, :, n] = A @ e_wv[b, :, n] + Bm @ h_w[b, :, n] + c[b]
with A = Wa@W_e2m, Bm = Wb@W_n2m, c[b] = Wa@b_e2m + Wb@b_n2m + Wc@nv[b] + b_resize.

Bottleneck: the per-core pool of 16 DMA engines (~25 GB/s each at >=4KB
packets, ~400 GB/s aggregate; HBM itself is not the limit). fp16 staging
(host casts, ~3e-4 normed rel err vs the 2e-2 gate) halves traffic vs fp32
to 30.7 MB/core (~75 us of pool time). Strided 2D reads (5KB rows, 40KB
stride) measured ~25% slower per engine than linear bursts, so the host
relays out both inputs as [BPC, 5, F, 4000] — every 4000-col piece is a
fully contiguous 1MB block (8000B rows) — and the kernel writes output
chunks to a linear [BPC, 10, F, 2000] layout that the host inverse-permutes.

A core's whole fp16 input (160 KB/partition) fits in SBUF, so ALL input
piece DMAs are issued up front into never-recycled tiles (e on the sync
hardware queue, h on the gpsimd queue), streaming with zero backpressure;
the PE then runs long uninterrupted bursts (its DVFS clock needs ~3 us of
continuous work to reach the fast p-state), two accumulated fp16 matmuls
per 500-col PSUM tile, A-passes of each 2000-col chunk first. PSUM bank
sets cycle 0-3/4-7 per chunk (8-buf pool, two chunks of decoupling). The
fp32 PSUM -> fp16 SBUF bias-add drain runs on VectorE (tensor_scalar_add,
~0.73 us per tile, off the critical path); per-chunk out DMAs trigger on
the dedicated scalar queue. Host upcasts the fp16 output to fp32.
Sharding: batch axis (16 batches -> 2 per core), zero comms.
"""

import sys

import numpy as np

try:
    from concourse import bacc, mybir
except ImportError:  # bare environment: fall back to the in-container repo
    sys.path.append("/opt/trn_rl_repo")
    from concourse import bacc, mybir
import concourse.tile as tile
from concourse.bass_utils import run_bass_kernel_spmd

B, F, N = 16, 128, 20000
NCORES = 8
BPC = B // NCORES          # batches per core
NT = 500                   # columns per matmul (fits one 2KB fp32 PSUM bank)
PC = 4000                  # columns per input DMA piece (8000B rows, linear)
NP = N // PC               # input pieces per batch (5)
CK = 2000                  # columns per output chunk (4 PSUM tiles)
NC = N // CK               # chunks per batch (10)

_cached_nc = None


def _build():
    global _cached_nc
    if _cached_nc is not None:
        return _cached_nc
    f32 = mybir.dt.float32
    f16 = mybir.dt.float16
    nc = bacc.Bacc("TRN2", target_bir_lowering=False, debug=False,
                   num_devices=NCORES)
    e_d = nc.dram_tensor("e_wv", (BPC, NP, F, PC), f16,
                         kind="ExternalInput").ap()
    h_d = nc.dram_tensor("h_w", (BPC, NP, F, PC), f16,
                         kind="ExternalInput").ap()
    at_d = nc.dram_tensor("at", (F, F), f16, kind="ExternalInput").ap()
    bt_d = nc.dram_tensor("bt", (F, F), f16, kind="ExternalInput").ap()
    c_d = nc.dram_tensor("c", (F, BPC), f32, kind="ExternalInput").ap()
    o_d = nc.dram_tensor("msg", (BPC, NC, F, CK), f16,
                         kind="ExternalOutput").ap()

    with tile.TileContext(nc) as tc:
        with tc.tile_pool(name="w", bufs=1) as wp, \
             tc.tile_pool(name="inv", bufs=BPC * NP) as inv, \
             tc.tile_pool(name="out", bufs=6) as opp, \
             tc.tile_pool(name="ps", bufs=8, space="PSUM") as psp:
            at_t = wp.tile([F, F], f16)
            nc.scalar.dma_start(at_t[:], at_d[:])
            bt_t = wp.tile([F, F], f16)
            nc.scalar.dma_start(bt_t[:], bt_d[:])
            c_t = wp.tile([F, BPC], f32)
            nc.scalar.dma_start(c_t[:], c_d[:])

            # whole-input prefetch: every piece gets its own SBUF tile slot
            # (bufs == piece count, so slots are never recycled and the DMA
            # streams run with zero backpressure).
            e_ts, h_ts = [], []
            for b in range(BPC):
                for p in range(NP):
                    e_t = inv.tile([F, PC], f16, tag="e")
                    nc.sync.dma_start(e_t[:], e_d[b, p])
                    e_ts.append(e_t)
                    h_t = inv.tile([F, PC], f16, tag="h")
                    nc.gpsimd.dma_start(h_t[:], h_d[b, p])
                    h_ts.append(h_t)

            for b in range(BPC):
                for c in range(NC):
                    o_t = opp.tile([F, CK], f16, tag="o")
                    ps_ts = []
                    for k in range(CK // NT):
                        t = c * (CK // NT) + k          # batch tile idx
                        g = b * NP + (t * NT) // PC     # global piece idx
                        ksl = slice((t * NT) % PC, (t * NT) % PC + NT)
                        ps_t = psp.tile([F, NT], f32, tag="ps")
                        ps_ts.append((ps_t, g, ksl))
                        nc.tensor.matmul(ps_t[:], at_t[:], e_ts[g][:, ksl],
                                         start=True, stop=False)
                    for k, (ps_t, g, ksl) in enumerate(ps_ts):
                        nc.tensor.matmul(ps_t[:], bt_t[:], h_ts[g][:, ksl],
                                         start=False, stop=True)
                        nc.vector.tensor_scalar_add(
                            o_t[:, k * NT:(k + 1) * NT], ps_t[:],
                            c_t[:, b:b + 1])
                    nc.scalar.dma_start(o_d[b, c], o_t[:])
    nc.finalize()
    _cached_nc = nc
    return nc


def _prepare_in_maps(h_w, h_v, e_wv, W_e2m, b_e2m, W_n2m, b_n2m,
                     W_resize, b_resize):
    f64 = np.float64
    M = F
    Wa = W_resize[:, :M].astype(f64)
    Wb = W_resize[:, M:2 * M].astype(f64)
    Wc = W_resize[:, 2 * M:].astype(f64)
    A = Wa @ W_e2m.astype(f64)
    Bm = Wb @ W_n2m.astype(f64)
    nv = h_v.astype(f64) @ W_n2m.astype(f64).T + b_n2m.astype(f64)
    c = (Wa @ b_e2m.astype(f64) + Wb @ b_n2m.astype(f64)
         + nv @ Wc.T + b_resize.astype(f64))          # [B, M]
    AT = np.ascontiguousarray(A.T).astype(np.float16)
    BT = np.ascontiguousarray(Bm.T).astype(np.float16)
    cT = np.ascontiguousarray(c.T).astype(np.float32)  # [M, B]

    # piece-linear layout: [B, F, N] -> [B, NP, F, PC] so each 4000-col
    # piece is one contiguous block in DRAM (pure linear DMA bursts).
    e16 = np.ascontiguousarray(
        e_wv.astype(np.float16).reshape(B, F, NP, PC).transpose(0, 2, 1, 3))
    h16 = np.ascontiguousarray(
        h_w.astype(np.float16).reshape(B, F, NP, PC).transpose(0, 2, 1, 3))
    in_maps = []
    for cid in range(NCORES):
        bs = slice(cid * BPC, (cid + 1) * BPC)
        in_maps.append({
            "e_wv": e16[bs],
            "h_w": h16[bs],
            "at": AT,
            "bt": BT,
            "c": np.ascontiguousarray(cT[:, bs]),
        })
    return in_maps


def kernel(**inputs):
    args = {k: np.asarray(inputs[k], dtype=np.float32)
            for k in ("h_w", "h_v", "e_wv", "W_e2m", "b_e2m", "W_n2m",
                      "b_n2m", "W_resize", "b_resize")}
    in_maps = _prepare_in_maps(**args)
    nc = _build()
    res = run_bass_kernel_spmd(nc, in_maps, core_ids=list(range(NCORES)))
    # [BPC, NC, F, CK] chunk-linear -> [BPC, F, N]
    out = np.concatenate(
        [np.asarray(r["msg"]).transpose(0, 2, 1, 3).reshape(BPC, F, N)
         for r in res.results], axis=0)
    return out.astype(np.float32)


# revision 17
# speedup vs baseline: 1.0586x; 1.0523x over previous
"""Trainium2 Bass kernel for nn_MessageFunctionForEvent (GNN message function).

Math: the reference is
    em  = W_e2m @ e_wv[b] + b_e2m          (per-node Linear on edge features)
    nw  = W_n2m @ h_w[b]  + b_n2m          (per-node Linear on node features)
    nv  = W_n2m @ h_v[b]  + b_n2m          (node-level, no n axis)
    msg = Wa @ em + Wb @ nw + (Wc @ nv + b_resize)[:, None]
which collapses (precomposing the tiny 128x128 weights on host) to
    msg[b, :, n] = A @ e_wv[b, :, n] + Bm @ h_w[b, :, n] + c[b]
with A = Wa@W_e2m, Bm = Wb@W_n2m, c[b] = Wa@b_e2m + Wb@b_n2m + Wc@nv[b] + b_resize.

Bottleneck: the per-core pool of 16 DMA engines (~23-25 GB/s each with
>=10KB packets, ~400 GB/s aggregate; a single queue tops out ~220 GB/s on
descriptor fetch). fp16 staging (host casts, ~3e-4 normed rel err vs the
2e-2 gate) halves traffic vs fp32 to 30.7 MB/core (~77 us of pool time).
Measured: natural strided rows (segments at 40KB stride) outperform a
host-relaid fully-linear layout under 3-stream contention, and bigger
packets win — so inputs stay in natural [BPC, F, N] layout and stream in
[2000, 8000, 8000, 2000]-col pieces (4-16KB row segments).

A core's whole fp16 input (160 KB/partition) fits in SBUF, so ALL input
piece DMAs are issued up front into never-recycled tiles (e on the sync
hardware queue, h on the gpsimd queue) — zero backpressure, the pool stays
fed. PE runs two accumulated fp16 matmuls per 500-col PSUM tile (8-bank
ring), VectorE drains fp32 PSUM -> fp16 SBUF with the bias via
tensor_scalar_add (646 ns/tile cadence, tracks input arrival). Out chunks
taper small first (out stream starts ~12 us) then 2500 cols (5KB packets),
triggered on the dedicated scalar queue; the final chunks round-robin
across all three queues (their input programs are exhausted by then) so the
out backlog flushes at full pool rate instead of tailing on one queue.
Host upcasts the fp16 output to fp32.
Sharding: batch axis (16 batches -> 2 per core), zero comms.
"""

import sys

import numpy as np

try:
    from concourse import bacc, mybir
except ImportError:  # bare environment: fall back to the in-container repo
    sys.path.append("/opt/trn_rl_repo")
    from concourse import bacc, mybir
import concourse.tile as tile
from concourse.bass_utils import run_bass_kernel_spmd

B, F, N = 16, 128, 20000
NCORES = 8
BPC = B // NCORES          # batches per core
NT = 500                   # columns per matmul (fits one 2KB fp32 PSUM bank)

PIECES = [2000, 8000, 8000, 2000]          # input DMA pieces per batch
CHUNKS0 = [500, 1000] + [2500] * 7 + [1000]      # out chunks, first batch
CHUNKS1 = [2500] * 7 + [1500, 1000]              # out chunks, last batch
TAIL_RR = 6                 # how many final out chunks round-robin queues

_cached_nc = None


def _build():
    global _cached_nc
    if _cached_nc is not None:
        return _cached_nc
    f32 = mybir.dt.float32
    f16 = mybir.dt.float16
    nc = bacc.Bacc("TRN2", target_bir_lowering=False, debug=False,
                   num_devices=NCORES)
    e_d = nc.dram_tensor("e_wv", (BPC, F, N), f16, kind="ExternalInput").ap()
    h_d = nc.dram_tensor("h_w", (BPC, F, N), f16, kind="ExternalInput").ap()
    at_d = nc.dram_tensor("at", (F, F), f16, kind="ExternalInput").ap()
    bt_d = nc.dram_tensor("bt", (F, F), f16, kind="ExternalInput").ap()
    c_d = nc.dram_tensor("c", (F, BPC), f32, kind="ExternalInput").ap()
    o_d = nc.dram_tensor("msg", (BPC, F, N), f16, kind="ExternalOutput").ap()

    chunks = {b: (CHUNKS0 if b == 0 else CHUNKS1) for b in range(BPC)}
    n_chunks = sum(len(v) for v in chunks.values())

    with tile.TileContext(nc) as tc:
        with tc.tile_pool(name="w", bufs=1) as wp, \
             tc.tile_pool(name="inv", bufs=1) as inv, \
             tc.tile_pool(name="out", bufs=5) as opp, \
             tc.tile_pool(name="ps", bufs=8, space="PSUM") as psp:
            at_t = wp.tile([F, F], f16)
            nc.scalar.dma_start(at_t[:], at_d[:])
            bt_t = wp.tile([F, F], f16)
            nc.scalar.dma_start(bt_t[:], bt_d[:])
            c_t = wp.tile([F, BPC], f32)
            nc.scalar.dma_start(c_t[:], c_d[:])

            # whole-input prefetch: every piece gets its own SBUF tile slot
            # (never recycled => the input streams run with no backpressure).
            # e_col[b] maps a column index to (piece tile, offset).
            nbufs = {pc: BPC * PIECES.count(pc) for pc in set(PIECES)}
            e_map, h_map = {}, {}
            for b in range(BPC):
                p0 = 0
                for pc in PIECES:
                    sl = slice(p0, p0 + pc)
                    e_t = inv.tile([F, pc], f16, tag=f"e{pc}",
                                   bufs=nbufs[pc], name=f"e_{b}_{p0}")
                    nc.sync.dma_start(e_t[:], e_d[b, :, sl])
                    h_t = inv.tile([F, pc], f16, tag=f"h{pc}",
                                   bufs=nbufs[pc], name=f"h_{b}_{p0}")
                    nc.gpsimd.dma_start(h_t[:], h_d[b, :, sl])
                    for col in range(p0, p0 + pc, NT):
                        e_map[(b, col)] = (e_t, col - p0)
                        h_map[(b, col)] = (h_t, col - p0)
                    p0 += pc

            ci = 0
            for b in range(BPC):
                n0 = 0
                for cs in chunks[b]:
                    o_t = opp.tile([F, cs], f16, tag="o")
                    ps_ts = []
                    for k in range(cs // NT):
                        col = n0 + k * NT
                        e_t, eo = e_map[(b, col)]
                        ps_t = psp.tile([F, NT], f32, tag="ps")
                        ps_ts.append((ps_t, col))
                        nc.tensor.matmul(ps_t[:], at_t[:],
                                         e_t[:, eo:eo + NT],
                                         start=True, stop=False)
                    for k, (ps_t, col) in enumerate(ps_ts):
                        h_t, ho = h_map[(b, col)]
                        nc.tensor.matmul(ps_t[:], bt_t[:],
                                         h_t[:, ho:ho + NT],
                                         start=False, stop=True)
                        nc.vector.tensor_scalar_add(
                            o_t[:, k * NT:(k + 1) * NT], ps_t[:],
                            c_t[:, b:b + 1])
                    if ci >= n_chunks - TAIL_RR:
                        oq = (nc.scalar, nc.sync, nc.gpsimd)[ci % 3]
                    else:
                        oq = nc.scalar
                    oq.dma_start(o_d[b, :, n0:n0 + cs], o_t[:])
                    ci += 1
                    n0 += cs
    nc.finalize()
    _cached_nc = nc
    return nc


def _prepare_in_maps(h_w, h_v, e_wv, W_e2m, b_e2m, W_n2m, b_n2m,
                     W_resize, b_resize):
    f64 = np.float64
    M = F
    Wa = W_resize[:, :M].astype(f64)
    Wb = W_resize[:, M:2 * M].astype(f64)
    Wc = W_resize[:, 2 * M:].astype(f64)
    A = Wa @ W_e2m.astype(f64)
    Bm = Wb @ W_n2m.astype(f64)
    nv = h_v.astype(f64) @ W_n2m.astype(f64).T + b_n2m.astype(f64)
    c = (Wa @ b_e2m.astype(f64) + Wb @ b_n2m.astype(f64)
         + nv @ Wc.T + b_resize.astype(f64))          # [B, M]
    AT = np.ascontiguousarray(A.T).astype(np.float16)
    BT = np.ascontiguousarray(Bm.T).astype(np.float16)
    cT = np.ascontiguousarray(c.T).astype(np.float32)  # [M, B]

    e16 = e_wv.astype(np.float16)
    h16 = h_w.astype(np.float16)
    in_maps = []
    for cid in range(NCORES):
        bs = slice(cid * BPC, (cid + 1) * BPC)
        in_maps.append({
            "e_wv": e16[bs],
            "h_w": h16[bs],
            "at": AT,
            "bt": BT,
            "c": np.ascontiguousarray(cT[:, bs]),
        })
    return in_maps


def kernel(**inputs):
    args = {k: np.asarray(inputs[k], dtype=np.float32)
            for k in ("h_w", "h_v", "e_wv", "W_e2m", "b_e2m", "W_n2m",
                      "b_n2m", "W_resize", "b_resize")}
    in_maps = _prepare_in_maps(**args)
    nc = _build()
    res = run_bass_kernel_spmd(nc, in_maps, core_ids=list(range(NCORES)))
    out = np.concatenate([r["msg"] for r in res.results], axis=0)
    return out.astype(np.float32)


# revision 18
# speedup vs baseline: 1.0953x; 1.0347x over previous
"""Trainium2 Bass kernel for nn_MessageFunctionForEvent (GNN message function).

Math: the reference is
    em  = W_e2m @ e_wv[b] + b_e2m          (per-node Linear on edge features)
    nw  = W_n2m @ h_w[b]  + b_n2m          (per-node Linear on node features)
    nv  = W_n2m @ h_v[b]  + b_n2m          (node-level, no n axis)
    msg = Wa @ em + Wb @ nw + (Wc @ nv + b_resize)[:, None]
which collapses (precomposing the tiny 128x128 weights on host) to
    msg[b, :, n] = A @ e_wv[b, :, n] + Bm @ h_w[b, :, n] + c[b]
with A = Wa@W_e2m, Bm = Wb@W_n2m, c[b] = Wa@b_e2m + Wb@b_n2m + Wc@nv[b] + b_resize.

Bottleneck: the per-core pool of 16 DMA engines (~23-25 GB/s each with
>=5KB packets; a single queue tops out ~180-220 GB/s on descriptor fetch,
so aggregate ~355 GB/s with two active queues, ~405-420 with three). fp16
staging (host casts, ~3e-4 normed rel err vs the 2e-2 gate) halves traffic
vs fp32 to 30.7 MB/core.

Device kernel: stream e/h 5000-col chunks (10KB strided row segments — the
best-measured packet shape) on the sync (e) and gpsimd (h) hardware queues,
double-buffered with a 4-deep tile ring; weights ride the scalar queue so h
starts immediately. Two accumulated 128x128 fp16 matmuls per 500-col fp32
PSUM tile (all A-passes of a chunk first — they only need e — then
B-passes as h lands), bias-add + fp16 cast via VectorE tensor_scalar_add
(PSUM->SBUF), outputs on the dedicated scalar queue. The first chunks are
small (2000/3000 cols) with half-chunk flushes so the out stream joins by
~14 us — with only two queues active the DMA pool idles ~12%. The last
batch tapers [3000,1000,500,500] so the pipeline drains in small quanta.
Sharding: batch axis (16 batches -> 2 per core), zero host re-layout.
"""

import sys

import numpy as np

try:
    from concourse import bacc, mybir
except ImportError:  # bare environment: fall back to the in-container repo
    sys.path.append("/opt/trn_rl_repo")
    from concourse import bacc, mybir
import concourse.tile as tile
from concourse.bass_utils import run_bass_kernel_spmd

B, F, N = 16, 128, 20000
NCORES = 8
BPC = B // NCORES          # batches per core
CH = 5000                  # columns per steady-state DMA chunk
NT = 500                   # columns per matmul (fits one 2KB fp32 PSUM bank)

_cached_nc = None


def _chunks_for(b):
    if b == 0:
        # ramp-up: small first chunks put the out stream on the wire early
        # (3 active queues beat 2 by ~50 GB/s of pool throughput)
        return [2000, 3000] + [CH] * 3
    taper = [1000, 500, 500]
    return [CH] * (N // CH - 1) + [CH - sum(taper)] + taper


def _build():
    global _cached_nc
    if _cached_nc is not None:
        return _cached_nc
    f32 = mybir.dt.float32
    f16 = mybir.dt.float16
    nc = bacc.Bacc("TRN2", target_bir_lowering=False, debug=False,
                   num_devices=NCORES)
    e_d = nc.dram_tensor("e_wv", (BPC, F, N), f16, kind="ExternalInput").ap()
    h_d = nc.dram_tensor("h_w", (BPC, F, N), f16, kind="ExternalInput").ap()
    at_d = nc.dram_tensor("at", (F, F), f16, kind="ExternalInput").ap()
    bt_d = nc.dram_tensor("bt", (F, F), f16, kind="ExternalInput").ap()
    c_d = nc.dram_tensor("c", (F, BPC), f32, kind="ExternalInput").ap()
    o_d = nc.dram_tensor("msg", (BPC, F, N), f16, kind="ExternalOutput").ap()

    with tile.TileContext(nc) as tc:
        with tc.tile_pool(name="w", bufs=1) as wp, \
             tc.tile_pool(name="eh", bufs=4) as ehp, \
             tc.tile_pool(name="out", bufs=3) as opp, \
             tc.tile_pool(name="ps", bufs=8, space="PSUM") as psp:
            at_t = wp.tile([F, F], f16)
            nc.scalar.dma_start(at_t[:], at_d[:])
            bt_t = wp.tile([F, F], f16)
            nc.scalar.dma_start(bt_t[:], bt_d[:])
            c_t = wp.tile([F, BPC], f32)
            nc.scalar.dma_start(c_t[:], c_d[:])
            for b in range(BPC):
                n0 = 0
                for cs in _chunks_for(b):
                    sl = slice(n0, n0 + cs)
                    e_t = ehp.tile([F, cs], f16, tag="e")
                    h_t = ehp.tile([F, cs], f16, tag="h")
                    o_t = opp.tile([F, cs], f16, tag="o")
                    nc.sync.dma_start(e_t[:], e_d[b, :, sl])
                    nc.gpsimd.dma_start(h_t[:], h_d[b, :, sl])
                    nk = cs // NT if cs >= NT else 1
                    nt = cs // nk
                    ps_ts = []
                    for k in range(nk):
                        ksl = slice(k * nt, (k + 1) * nt)
                        ps_t = psp.tile([F, nt], f32, tag="ps")
                        ps_ts.append(ps_t)
                        nc.tensor.matmul(ps_t[:], at_t[:], e_t[:, ksl],
                                         start=True, stop=False)
                    for k in range(nk):
                        ksl = slice(k * nt, (k + 1) * nt)
                        nc.tensor.matmul(ps_ts[k][:], bt_t[:], h_t[:, ksl],
                                         start=False, stop=True)
                        nc.vector.tensor_scalar_add(o_t[:, ksl], ps_ts[k][:],
                                                    c_t[:, b:b + 1])
                        # stream the first half of the chunk out as soon as
                        # its bias-adds are done (halves drain latency)
                        if nk >= 4 and k == nk // 2 - 1:
                            nc.scalar.dma_start(o_d[b, :, n0:n0 + nt * (nk // 2)],
                                                o_t[:, :nt * (nk // 2)])
                    lo = nt * (nk // 2) if nk >= 4 else 0
                    nc.scalar.dma_start(o_d[b, :, n0 + lo:n0 + cs],
                                        o_t[:, lo:])
                    n0 += cs
    nc.finalize()
    _cached_nc = nc
    return nc


def _prepare_in_maps(h_w, h_v, e_wv, W_e2m, b_e2m, W_n2m, b_n2m,
                     W_resize, b_resize):
    f64 = np.float64
    M = F
    Wa = W_resize[:, :M].astype(f64)
    Wb = W_resize[:, M:2 * M].astype(f64)
    Wc = W_resize[:, 2 * M:].astype(f64)
    A = Wa @ W_e2m.astype(f64)
    Bm = Wb @ W_n2m.astype(f64)
    nv = h_v.astype(f64) @ W_n2m.astype(f64).T + b_n2m.astype(f64)
    c = (Wa @ b_e2m.astype(f64) + Wb @ b_n2m.astype(f64)
         + nv @ Wc.T + b_resize.astype(f64))          # [B, M]
    AT = np.ascontiguousarray(A.T).astype(np.float16)
    BT = np.ascontiguousarray(Bm.T).astype(np.float16)
    cT = np.ascontiguousarray(c.T).astype(np.float32)  # [M, B]

    e16 = e_wv.astype(np.float16)
    h16 = h_w.astype(np.float16)
    in_maps = []
    for cid in range(NCORES):
        bs = slice(cid * BPC, (cid + 1) * BPC)
        in_maps.append({
            "e_wv": e16[bs],
            "h_w": h16[bs],
            "at": AT,
            "bt": BT,
            "c": np.ascontiguousarray(cT[:, bs]),
        })
    return in_maps


def kernel(**inputs):
    args = {k: np.asarray(inputs[k], dtype=np.float32)
            for k in ("h_w", "h_v", "e_wv", "W_e2m", "b_e2m", "W_n2m",
                      "b_n2m", "W_resize", "b_resize")}
    in_maps = _prepare_in_maps(**args)
    nc = _build()
    res = run_bass_kernel_spmd(nc, in_maps, core_ids=list(range(NCORES)))
    out = np.concatenate([r["msg"] for r in res.results], axis=0)
    return out.astype(np.float32)
